# revision 8
# baseline (speedup 1.0000x reference)
"""Single-head causal attention on 8 TRN2 NeuronCores.

Reference computation (per batch b):
    q = emb @ Wq; k = emb @ Wk; v = emb @ Wv      # [T, H]
    S = q @ k.T / sqrt(H)  (causal masked)
    out = softmax(S) @ v                          # [T, H]

Shapes: B=16, T=2048, E=1024, H=64, fp32 in/out.

Sharding: pure data-parallel over batch - 2 batches per core, no
collectives.  Each core runs the identical graph on its own shard.

Compute dtype: bf16 operands with fp32 PSUM accumulation (full-rate
TensorEngine; plain fp32 matmul is 4 cycles/row).  Inputs are cast
f32->bf16 during the DMA (SWDGE).  Final normalization is fp32.

Per-core algorithm (layouts chosen so the only big transpose is the
unavoidable embedding transpose, done on the TensorEngine):

  1. embT tiles: PE-transpose emb [128t,128e] tiles.
  2. Packed projection: lhsT=[Wq|Wk] chunk -> psum rows 0:64 = Q^T,
     64:128 = K^T (the [H, T] layout S^T needs).  V^T likewise, then 16
     small PE transposes -> V natural [T, H+1] with a ones column
     (normalizer trick).
  3. S^T[k,q] = matmul(lhsT=K^T tile, rhs=Q^T chunk).  exp via ScalarE
     (scale=1/8 folded in; no max-subtraction: scores are O(+-15) so
     fp32 exp is safe).  Causal masking multiplies the 4 diagonal-band
     tiles with precomputed 0/1 masks.
  4. O'^T[h|1, q] += matmul(lhsT=V' tile [k,65], rhs=A^T [k,q]):
     row 64 accumulates the softmax denominator.
  5. 4 small fp32 PE transposes per q-chunk -> O' [q, 65]; DVE
     reciprocal of col 64, tensor_scalar_mul -> out tile; DMA out.
"""

import os
import sys

sys.path.insert(0, "/opt/trn_rl_repo")

import numpy as np

import concourse.bass as bass
import concourse.bacc as bacc
import concourse.mybir as mybir
import concourse.tile as tile
from concourse.masks import make_identity

B, T, E, H = 16, 2048, 1024, 64
NCORES = 8
BP = B // NCORES          # batches per core
TQ = 512                  # q-chunk width (free dim of the big matmuls)
NTC = T // TQ             # 4 q/t chunks
NKT = T // 128            # 16 k/t tiles
NEC = E // 128            # 8 e chunks
F32 = mybir.dt.float32
BF16 = mybir.dt.bfloat16


def build_nc():
    nc = bacc.Bacc(None)
    emb = nc.declare_dram_parameter("embeddings", [BP, T, E], F32, isOutput=False)
    wq = nc.declare_dram_parameter("Wq", [E, H], F32, isOutput=False)
    wk = nc.declare_dram_parameter("Wk", [E, H], F32, isOutput=False)
    wv = nc.declare_dram_parameter("Wv", [E, H], F32, isOutput=False)
    out = nc.declare_dram_parameter("out", [BP, T, H], F32, isOutput=True)

    with tile.TileContext(nc) as tc:
        with (
            tc.tile_pool(name="const", bufs=1) as const,
            tc.tile_pool(name="nat", bufs=6) as nat_pool,
            tc.tile_pool(name="embT", bufs=2) as embT_pool,
            tc.tile_pool(name="qkt", bufs=2) as qkt_pool,
            tc.tile_pool(name="vt", bufs=2) as vt_pool,
            tc.tile_pool(name="vp", bufs=2) as vp_pool,
            tc.tile_pool(name="at", bufs=4) as at_pool,
            tc.tile_pool(name="osb", bufs=2) as osb_pool,
            tc.tile_pool(name="stage", bufs=2) as stage_pool,
            tc.tile_pool(name="rec", bufs=4) as rec_pool,
            tc.tile_pool(name="psA", bufs=3, space="PSUM") as psA,
            tc.tile_pool(name="psT", bufs=2, space="PSUM") as psT,
            tc.tile_pool(name="psB", bufs=2, space="PSUM") as psB,
        ):
            # ---- constants ----
            ident = const.tile([128, 128], BF16)
            make_identity(nc, ident)
            ident32 = const.tile([128, 128], F32)
            make_identity(nc, ident32)

            # diagonal-band causal masks for S^T[k, q] tiles:
            # M_j[kk, qq] = 1 iff kk + 128*j <= qq   (j = k-tile index
            # within the 512-wide q-chunk)
            masks = []
            for j in range(TQ // 128):
                m = const.tile([128, TQ], BF16, name=f"mask{j}")
                nc.gpsimd.memset(m, 1.0)
                nc.gpsimd.affine_select(
                    out=m,
                    in_=m,
                    compare_op=mybir.AluOpType.is_ge,
                    fill=0.0,
                    base=-128 * j,
                    channel_multiplier=-1,
                    pattern=[[1, TQ]],
                )
                masks.append(m)

            # weights (cast f32->bf16 during DMA): [Wq|Wk] packed per
            # e-chunk -> [128, 8*128]; Wv -> [128, 8*64]
            wqk_sb = const.tile([128, NEC * 128], BF16)
            wv_sb = const.tile([128, NEC * H], BF16)
            for ec in range(NEC):
                es = slice(ec * 128, (ec + 1) * 128)
                nc.gpsimd.dma_start(
                    out=wqk_sb[:, ec * 128 : ec * 128 + H], in_=wq[es, :]
                )
                nc.gpsimd.dma_start(
                    out=wqk_sb[:, ec * 128 + H : (ec + 1) * 128], in_=wk[es, :]
                )
                nc.gpsimd.dma_start(out=wv_sb[:, ec * H : (ec + 1) * H], in_=wv[es, :])

            for b in range(BP):
                # ---- phase 1: embT + projections ----
                qt_sb = qkt_pool.tile([H, T], BF16, name=f"qt{b}", tag="qt")
                kt_sb = qkt_pool.tile([H, T], BF16, name=f"kt{b}", tag="kt")
                vt_sb = vt_pool.tile([H, T], BF16, name=f"vt{b}", tag="vt")

                for tc_i in range(NTC):
                    nats = []
                    for tt in range(TQ // 128):
                        t0 = tc_i * TQ + tt * 128
                        nt = nat_pool.tile([128, E], BF16, name=f"nat{b}_{tc_i}_{tt}", tag="nat")
                        nc.gpsimd.dma_start(out=nt, in_=emb[b, t0 : t0 + 128, :])
                        nats.append(nt)

                    embT = embT_pool.tile([128, NEC * TQ], BF16, name=f"embT{b}_{tc_i}", tag="embT")
                    for ec in range(NEC):
                        ps = psT.tile([128, TQ], BF16, name=f"ps_tr{b}_{tc_i}_{ec}", tag="T")
                        for tt in range(TQ // 128):
                            nc.tensor.transpose(
                                out=ps[:, tt * 128 : (tt + 1) * 128],
                                in_=nats[tt][:, ec * 128 : (ec + 1) * 128],
                                identity=ident,
                            )
                        nc.vector.tensor_copy(embT[:, ec * TQ : (ec + 1) * TQ], ps)

                    cs = slice(tc_i * TQ, (tc_i + 1) * TQ)
                    psqk = psA.tile([128, TQ], F32, name=f"ps_qk{b}_{tc_i}", tag="A")
                    for ec in range(NEC):
                        nc.tensor.matmul(
                            out=psqk,
                            lhsT=wqk_sb[:, ec * 128 : (ec + 1) * 128],
                            rhs=embT[:, ec * TQ : (ec + 1) * TQ],
                            start=(ec == 0),
                            stop=(ec == NEC - 1),
                        )
                    nc.vector.tensor_copy(qt_sb[:, cs], psqk[0:H, :])
                    nc.vector.tensor_copy(kt_sb[:, cs], psqk[H:128, :])

                    psv = psB.tile([H, TQ], F32, name=f"ps_v{b}_{tc_i}", tag="B")
                    for ec in range(NEC):
                        nc.tensor.matmul(
                            out=psv,
                            lhsT=wv_sb[:, ec * H : (ec + 1) * H],
                            rhs=embT[:, ec * TQ : (ec + 1) * TQ],
                            start=(ec == 0),
                            stop=(ec == NEC - 1),
                        )
                    nc.vector.tensor_copy(vt_sb[:, cs], psv)

                # ---- phase 1b: V' = [V | 1] in natural [T, H+1] layout ----
                vp_sb = vp_pool.tile([128, NKT * (H + 1)], BF16, name=f"vp{b}", tag="vp")
                nc.vector.memset(vp_sb, 1.0)
                for kt in range(NKT):
                    psvp = psT.tile([128, H], BF16, name=f"ps_vp{b}_{kt}", tag="T")
                    nc.tensor.transpose(
                        out=psvp,
                        in_=vt_sb[:, kt * 128 : (kt + 1) * 128],
                        identity=ident[0:H, 0:H],
                    )
                    nc.vector.tensor_copy(
                        vp_sb[:, kt * (H + 1) : kt * (H + 1) + H], psvp
                    )

                # ---- phase 2: attention per q-chunk ----
                for qc in range(NTC):
                    qs = slice(qc * TQ, (qc + 1) * TQ)
                    nkb = 4 * qc + 4
                    pso = psB.tile([H + 1, TQ], F32, name=f"ps_o{b}_{qc}", tag="B")
                    for kb in range(nkb):
                        pss = psA.tile([128, TQ], F32, name=f"ps_s{b}_{qc}_{kb}", tag="A")
                        nc.tensor.matmul(
                            out=pss,
                            lhsT=kt_sb[:, kb * 128 : (kb + 1) * 128],
                            rhs=qt_sb[:, qs],
                            start=True,
                            stop=True,
                            skip_group_check=True,
                        )
                        at = at_pool.tile([128, TQ], BF16, name=f"at{b}_{qc}_{kb}", tag="at")
                        nc.scalar.activation(
                            out=at,
                            in_=pss,
                            func=mybir.ActivationFunctionType.Exp,
                            scale=0.125,
                        )
                        if kb >= 4 * qc:
                            nc.vector.tensor_mul(at, at, masks[kb - 4 * qc])
                        nc.tensor.matmul(
                            out=pso,
                            lhsT=vp_sb[:, kb * (H + 1) : (kb + 1) * (H + 1)],
                            rhs=at,
                            start=(kb == 0),
                            stop=(kb == nkb - 1),
                            skip_group_check=True,
                        )

                    osb = osb_pool.tile([H + 1, TQ], F32, name=f"osb{b}_{qc}", tag="osb")
                    nc.vector.tensor_copy(osb, pso)
                    stg = stage_pool.tile([128, 4 * H], F32, name=f"stg{b}_{qc}", tag="stg")
                    for i in range(TQ // 128):
                        pst = psB.tile([128, H + 1], F32, name=f"ps_t{b}_{qc}_{i}", tag="B")
                        nc.tensor.transpose(
                            out=pst,
                            in_=osb[:, i * 128 : (i + 1) * 128],
                            identity=ident32[0 : H + 1, 0 : H + 1],
                        )
                        rec = rec_pool.tile([128, 1], F32, name=f"rec{b}_{qc}_{i}", tag="rec")
                        nc.vector.reciprocal(rec, pst[:, H : H + 1])
                        nc.vector.tensor_scalar_mul(
                            stg[:, i * H : (i + 1) * H], pst[:, 0:H], rec
                        )
                    out_view = out[b, qs, :].rearrange("(i p) h -> p i h", p=128)
                    stg_view = stg[:, :].rearrange("p (i h) -> p i h", h=H)
                    nc.sync.dma_start(out=out_view, in_=stg_view)

    nc.finalize()
    return nc


_NC = None
LAST_RESULTS = None


def _ensure_profile_hook():
    """Recreate the antenv.axon_hooks NTFF-profile registry if the image
    lacks it (the boot degrades silently in that case), driving profiling
    via ctypes into libaxon_pjrt.so."""
    import contextlib
    import ctypes
    import types

    try:
        from antenv.axon_hooks import get_axon_ntff_profile_hook  # noqa: F401

        return
    except ImportError:
        pass

    import antenv

    mod = types.ModuleType("antenv.axon_hooks")
    _box = [None]
    mod.set_axon_ntff_profile_hook = lambda h: _box.__setitem__(0, h)
    mod.get_axon_ntff_profile_hook = lambda: _box[0]
    sys.modules["antenv.axon_hooks"] = mod
    antenv.axon_hooks = mod

    so_path = "/opt/axon/libaxon_pjrt.so"
    try:
        lib = ctypes.CDLL(so_path)
    except OSError:
        return
    if not hasattr(lib, "axon_start_nrt_profile"):
        return
    lib.axon_start_nrt_profile.argtypes = [
        ctypes.POINTER(ctypes.c_int64),
        ctypes.c_size_t,
    ]
    lib.axon_start_nrt_profile.restype = ctypes.c_int64
    lib.axon_stop_nrt_profile.argtypes = [ctypes.c_char_p]
    lib.axon_stop_nrt_profile.restype = ctypes.c_int64

    @contextlib.contextmanager
    def _hook(output_dir, device_ids):
        import jax

        jax.devices()
        if device_ids:
            ids = (ctypes.c_int64 * len(device_ids))(*device_ids)
            rc = lib.axon_start_nrt_profile(ids, len(device_ids))
        else:
            rc = lib.axon_start_nrt_profile(None, 0)
        if rc != 0:
            raise RuntimeError(f"axon_start_nrt_profile rc={rc}")
        try:
            yield
        finally:
            n = lib.axon_stop_nrt_profile(str(output_dir).encode())
            print(f"profile: {n} file(s) written to {output_dir}")

    mod.set_axon_ntff_profile_hook(_hook)

    # upload_artifacts needs bucket credentials this container lacks.
    import concourse.bass_utils as _bu

    _bu.upload_artifacts = lambda tmpdir: f"local:{tmpdir}"


def kernel(embeddings, Wq, Wk, Wv):
    global _NC, LAST_RESULTS
    from concourse.bass_utils import run_bass_kernel_spmd

    if bool(int(os.environ.get("KERNEL_TRACE", "0"))):
        _ensure_profile_hook()

    embeddings = np.ascontiguousarray(embeddings, dtype=np.float32)
    Wq = np.ascontiguousarray(Wq, dtype=np.float32)
    Wk = np.ascontiguousarray(Wk, dtype=np.float32)
    Wv = np.ascontiguousarray(Wv, dtype=np.float32)

    if _NC is None:
        _NC = build_nc()

    in_maps = []
    for c in range(NCORES):
        in_maps.append(
            {
                "embeddings": embeddings[c * BP : (c + 1) * BP],
                "Wq": Wq,
                "Wk": Wk,
                "Wv": Wv,
            }
        )
    res = run_bass_kernel_spmd(
        _NC,
        in_maps,
        core_ids=list(range(NCORES)),
        trace=bool(int(os.environ.get("KERNEL_TRACE", "0"))),
    )
    LAST_RESULTS = res
    out = np.concatenate([res.results[c]["out"] for c in range(NCORES)], axis=0)
    return out


# revision 11
# speedup vs baseline: 1.0261x; 1.0261x over previous
"""Single-head causal attention on 8 TRN2 NeuronCores.

Reference computation (per batch b):
    q = emb @ Wq; k = emb @ Wk; v = emb @ Wv      # [T, H]
    S = q @ k.T / sqrt(H)  (causal masked)
    out = softmax(S) @ v                          # [T, H]

Shapes: B=16, T=2048, E=1024, H=64, fp32 in/out.

Sharding: pure data-parallel over batch - 2 batches per core, no
collectives.  Each core runs the identical graph on its own shard.

Compute dtype: bf16 operands with fp32 PSUM accumulation (full-rate
TensorEngine; plain fp32 matmul is 4 cycles/row).  Inputs are cast
f32->bf16 during the DMA (SWDGE).  Final normalization is fp32.

Per-core algorithm (layouts chosen so the only big transpose is the
unavoidable embedding transpose, done on the TensorEngine):

  1. embT tiles: PE-transpose emb [128t,128e] tiles.
  2. Packed projection: lhsT=[Wq|Wk] chunk -> psum rows 0:64 = Q^T,
     64:128 = K^T (the [H, T] layout S^T needs).  V^T likewise, then 16
     small PE transposes -> V natural [T, H+1] with a ones column
     (normalizer trick).
  3. S^T[k,q] = matmul(lhsT=K^T tile, rhs=Q^T chunk).  exp via ScalarE
     (scale=1/8 folded in; no max-subtraction: scores are O(+-15) so
     fp32 exp is safe).  Causal masking multiplies the 4 diagonal-band
     tiles with precomputed 0/1 masks.
  4. O'^T[h|1, q] += matmul(lhsT=V' tile [k,65], rhs=A^T [k,q]):
     row 64 accumulates the softmax denominator.
  5. 4 small fp32 PE transposes per q-chunk -> O' [q, 65]; DVE
     reciprocal of col 64, tensor_scalar_mul -> out tile; DMA out.
"""

import os
import sys

sys.path.insert(0, "/opt/trn_rl_repo")

import numpy as np

import concourse.bass as bass
import concourse.bacc as bacc
import concourse.mybir as mybir
import concourse.tile as tile
from concourse.masks import make_identity

B, T, E, H = 16, 2048, 1024, 64
NCORES = 8
BP = B // NCORES          # batches per core
TQ = 512                  # q-chunk width (free dim of the big matmuls)
NTC = T // TQ             # 4 q/t chunks
NKT = T // 128            # 16 k/t tiles
NEC = E // 128            # 8 e chunks
F32 = mybir.dt.float32
BF16 = mybir.dt.bfloat16


def build_nc():
    nc = bacc.Bacc(None)
    emb = nc.declare_dram_parameter("embeddings", [BP, T, E], F32, isOutput=False)
    wq = nc.declare_dram_parameter("Wq", [E, H], F32, isOutput=False)
    wk = nc.declare_dram_parameter("Wk", [E, H], F32, isOutput=False)
    wv = nc.declare_dram_parameter("Wv", [E, H], F32, isOutput=False)
    out = nc.declare_dram_parameter("out", [BP, T, H], F32, isOutput=True)

    with tile.TileContext(nc) as tc:
        with (
            tc.tile_pool(name="const", bufs=1) as const,
            tc.tile_pool(name="nat", bufs=3) as nat_pool,
            tc.tile_pool(name="embT", bufs=2) as embT_pool,
            tc.tile_pool(name="qkt", bufs=2) as qkt_pool,
            tc.tile_pool(name="vt", bufs=2) as vt_pool,
            tc.tile_pool(name="vp", bufs=2) as vp_pool,
            tc.tile_pool(name="at", bufs=6) as at_pool,
            tc.tile_pool(name="osb", bufs=2) as osb_pool,
            tc.tile_pool(name="stage", bufs=2) as stage_pool,
            tc.tile_pool(name="rec", bufs=4) as rec_pool,
            tc.tile_pool(name="psA", bufs=4, space="PSUM") as psA,
            tc.tile_pool(name="psB", bufs=3, space="PSUM") as psB,
        ):
            # ---- constants ----
            ident = const.tile([128, 128], BF16)
            make_identity(nc, ident)
            ident32 = const.tile([128, 128], F32)
            make_identity(nc, ident32)

            # causal mask for the diagonal-band S^T tiles: with the
            # q-range of each diagonal matmul starting at global q =
            # 128*kb, validity is simply kk <= qq.
            mask0 = const.tile([128, TQ], BF16)
            nc.gpsimd.memset(mask0, 1.0)
            nc.gpsimd.affine_select(
                out=mask0,
                in_=mask0,
                compare_op=mybir.AluOpType.is_ge,
                fill=0.0,
                base=0,
                channel_multiplier=-1,
                pattern=[[1, TQ]],
            )

            # weights (cast f32->bf16 during DMA): [Wq|Wk] packed per
            # e-chunk -> [128, 8*128]; Wv -> [128, 8*64]
            wqk_sb = const.tile([128, NEC * 128], BF16)
            wv_sb = const.tile([128, NEC * H], BF16)
            wqk_v = wqk_sb[:, :].rearrange("p (ec c) -> p ec c", c=128)
            nc.gpsimd.dma_start(
                out=wqk_v[:, :, 0:H], in_=wq.rearrange("(ec p) h -> p ec h", p=128)
            )
            nc.gpsimd.dma_start(
                out=wqk_v[:, :, H:128], in_=wk.rearrange("(ec p) h -> p ec h", p=128)
            )
            nc.gpsimd.dma_start(
                out=wv_sb[:, :].rearrange("p (ec c) -> p ec c", c=H),
                in_=wv.rearrange("(ec p) h -> p ec h", p=128),
            )

            for b in range(BP):
                # ---- phase 1: embT + projections ----
                qt_sb = qkt_pool.tile([H, T], BF16, name=f"qt{b}", tag="qt")
                kt_sb = qkt_pool.tile([H, T], BF16, name=f"kt{b}", tag="kt")
                vt_sb = vt_pool.tile([H, T], BF16, name=f"vt{b}", tag="vt")

                for tc_i in range(NTC):
                    # one 2 MB cast-DMA per 512-token chunk
                    nat = nat_pool.tile([128, TQ // 128, E], BF16, name=f"nat{b}_{tc_i}", tag="nat")
                    nc.gpsimd.dma_start(
                        out=nat,
                        in_=emb[b, tc_i * TQ : (tc_i + 1) * TQ, :].rearrange(
                            "(tt p) e -> p tt e", p=128
                        ),
                    )

                    embT = embT_pool.tile([128, NEC * TQ], BF16, name=f"embT{b}_{tc_i}", tag="embT")
                    for ec in range(NEC):
                        # transpose as a REGULAR matmul against identity
                        # (is_transpose doesn't count as PE activity for
                        # the HAM clock gate and would leave it cold)
                        ps = psA.tile([128, TQ], F32, name=f"ps_tr{b}_{tc_i}_{ec}", tag="A")
                        for tt in range(TQ // 128):
                            nc.tensor.matmul(
                                out=ps[:, tt * 128 : (tt + 1) * 128],
                                lhsT=nat[:, tt, ec * 128 : (ec + 1) * 128],
                                rhs=ident,
                                start=True,
                                stop=True,
                            )
                        nc.vector.tensor_copy(embT[:, ec * TQ : (ec + 1) * TQ], ps)

                    cs = slice(tc_i * TQ, (tc_i + 1) * TQ)
                    psqk = psA.tile([128, TQ], F32, name=f"ps_qk{b}_{tc_i}", tag="A")
                    for ec in range(NEC):
                        nc.tensor.matmul(
                            out=psqk,
                            lhsT=wqk_sb[:, ec * 128 : (ec + 1) * 128],
                            rhs=embT[:, ec * TQ : (ec + 1) * TQ],
                            start=(ec == 0),
                            stop=(ec == NEC - 1),
                        )
                    nc.vector.tensor_copy(qt_sb[:, cs], psqk[0:H, :])
                    nc.vector.tensor_copy(kt_sb[:, cs], psqk[H:128, :])

                    psv = psB.tile([H, TQ], F32, name=f"ps_v{b}_{tc_i}", tag="B")
                    for ec in range(NEC):
                        nc.tensor.matmul(
                            out=psv,
                            lhsT=wv_sb[:, ec * H : (ec + 1) * H],
                            rhs=embT[:, ec * TQ : (ec + 1) * TQ],
                            start=(ec == 0),
                            stop=(ec == NEC - 1),
                        )
                    nc.vector.tensor_copy(vt_sb[:, cs], psv)

                # ---- phase 1b: V' = [V | 1] in natural [T, H+1] layout ----
                vp_sb = vp_pool.tile([128, NKT * (H + 1)], BF16, name=f"vp{b}", tag="vp")
                nc.vector.memset(vp_sb, 1.0)
                for kt in range(NKT):
                    psvp = psB.tile([128, H], F32, name=f"ps_vp{b}_{kt}", tag="B")
                    nc.tensor.matmul(
                        out=psvp,
                        lhsT=vt_sb[:, kt * 128 : (kt + 1) * 128],
                        rhs=ident[0:H, 0:H],
                        start=True,
                        stop=True,
                    )
                    nc.vector.tensor_copy(
                        vp_sb[:, kt * (H + 1) : kt * (H + 1) + H], psvp
                    )

                # ---- phase 2: attention per q-chunk ----
                for qc in range(NTC):
                    nkb = 4 * qc + 4
                    pso = psB.tile([H + 1, TQ], F32, name=f"ps_o{b}_{qc}", tag="B")
                    for kb in range(nkb):
                        # diagonal-band tiles only need q >= 128*kb: shrink
                        # the moving operand (off = local q offset)
                        j = kb - 4 * qc
                        off = 128 * j if j >= 0 else 0
                        n = TQ - off
                        q0 = qc * TQ + off
                        pss = psA.tile([128, TQ], F32, name=f"ps_s{b}_{qc}_{kb}", tag="A")
                        nc.tensor.matmul(
                            out=pss[:, 0:n],
                            lhsT=kt_sb[:, kb * 128 : (kb + 1) * 128],
                            rhs=qt_sb[:, q0 : q0 + n],
                            start=True,
                            stop=True,
                            skip_group_check=True,
                        )
                        at = at_pool.tile([128, TQ], BF16, name=f"at{b}_{qc}_{kb}", tag="at")
                        nc.scalar.activation(
                            out=at[:, 0:n],
                            in_=pss[:, 0:n],
                            func=mybir.ActivationFunctionType.Exp,
                            scale=0.125,
                        )
                        if j >= 0:
                            nc.vector.tensor_mul(
                                at[:, 0:n], at[:, 0:n], mask0[:, 0:n]
                            )
                        nc.tensor.matmul(
                            out=pso[:, off:TQ],
                            lhsT=vp_sb[:, kb * (H + 1) : (kb + 1) * (H + 1)],
                            rhs=at[:, 0:n],
                            start=(kb == 0),
                            stop=(kb == nkb - 1),
                            skip_group_check=True,
                        )

                    qs = slice(qc * TQ, (qc + 1) * TQ)
                    osb = osb_pool.tile([H + 1, TQ], F32, name=f"osb{b}_{qc}", tag="osb")
                    nc.vector.tensor_copy(osb, pso)
                    stg = stage_pool.tile([128, 4 * H], F32, name=f"stg{b}_{qc}", tag="stg")
                    for i in range(TQ // 128):
                        pst = psB.tile([128, H + 1], F32, name=f"ps_t{b}_{qc}_{i}", tag="B")
                        nc.tensor.transpose(
                            out=pst,
                            in_=osb[:, i * 128 : (i + 1) * 128],
                            identity=ident32[0 : H + 1, 0 : H + 1],
                        )
                        rec = rec_pool.tile([128, 1], F32, name=f"rec{b}_{qc}_{i}", tag="rec")
                        nc.vector.reciprocal(rec, pst[:, H : H + 1])
                        nc.vector.tensor_scalar_mul(
                            stg[:, i * H : (i + 1) * H], pst[:, 0:H], rec
                        )
                    out_view = out[b, qs, :].rearrange("(i p) h -> p i h", p=128)
                    stg_view = stg[:, :].rearrange("p (i h) -> p i h", h=H)
                    nc.sync.dma_start(out=out_view, in_=stg_view)

    nc.finalize()
    return nc


_NC = None
LAST_RESULTS = None


def _ensure_profile_hook():
    """Recreate the antenv.axon_hooks NTFF-profile registry if the image
    lacks it (the boot degrades silently in that case), driving profiling
    via ctypes into libaxon_pjrt.so."""
    import contextlib
    import ctypes
    import types

    try:
        from antenv.axon_hooks import get_axon_ntff_profile_hook  # noqa: F401

        return
    except ImportError:
        pass

    import antenv

    mod = types.ModuleType("antenv.axon_hooks")
    _box = [None]
    mod.set_axon_ntff_profile_hook = lambda h: _box.__setitem__(0, h)
    mod.get_axon_ntff_profile_hook = lambda: _box[0]
    sys.modules["antenv.axon_hooks"] = mod
    antenv.axon_hooks = mod

    so_path = "/opt/axon/libaxon_pjrt.so"
    try:
        lib = ctypes.CDLL(so_path)
    except OSError:
        return
    if not hasattr(lib, "axon_start_nrt_profile"):
        return
    lib.axon_start_nrt_profile.argtypes = [
        ctypes.POINTER(ctypes.c_int64),
        ctypes.c_size_t,
    ]
    lib.axon_start_nrt_profile.restype = ctypes.c_int64
    lib.axon_stop_nrt_profile.argtypes = [ctypes.c_char_p]
    lib.axon_stop_nrt_profile.restype = ctypes.c_int64

    @contextlib.contextmanager
    def _hook(output_dir, device_ids):
        import jax

        jax.devices()
        if device_ids:
            ids = (ctypes.c_int64 * len(device_ids))(*device_ids)
            rc = lib.axon_start_nrt_profile(ids, len(device_ids))
        else:
            rc = lib.axon_start_nrt_profile(None, 0)
        if rc != 0:
            raise RuntimeError(f"axon_start_nrt_profile rc={rc}")
        try:
            yield
        finally:
            n = lib.axon_stop_nrt_profile(str(output_dir).encode())
            print(f"profile: {n} file(s) written to {output_dir}")

    mod.set_axon_ntff_profile_hook(_hook)

    # upload_artifacts needs bucket credentials this container lacks.
    import concourse.bass_utils as _bu

    _bu.upload_artifacts = lambda tmpdir: f"local:{tmpdir}"


def kernel(embeddings, Wq, Wk, Wv):
    global _NC, LAST_RESULTS
    from concourse.bass_utils import run_bass_kernel_spmd

    if bool(int(os.environ.get("KERNEL_TRACE", "0"))):
        _ensure_profile_hook()

    embeddings = np.ascontiguousarray(embeddings, dtype=np.float32)
    Wq = np.ascontiguousarray(Wq, dtype=np.float32)
    Wk = np.ascontiguousarray(Wk, dtype=np.float32)
    Wv = np.ascontiguousarray(Wv, dtype=np.float32)

    if _NC is None:
        _NC = build_nc()

    in_maps = []
    for c in range(NCORES):
        in_maps.append(
            {
                "embeddings": embeddings[c * BP : (c + 1) * BP],
                "Wq": Wq,
                "Wk": Wk,
                "Wv": Wv,
            }
        )
    res = run_bass_kernel_spmd(
        _NC,
        in_maps,
        core_ids=list(range(NCORES)),
        trace=bool(int(os.environ.get("KERNEL_TRACE", "0"))),
    )
    LAST_RESULTS = res
    out = np.concatenate([res.results[c]["out"] for c in range(NCORES)], axis=0)
    return out
